# revision 19
# baseline (speedup 1.0000x reference)
"""Causal self-attention (B=4, S=2048, D=1024, single 1024-wide head) on 8 TRN2 cores.

Sharding: core c -> batch b=c//2, parity h=c%2. Core h projects K/V only for
its parity key tiles (t%2==h, 8 tiles — half the work) and computes PARTIAL
attention for ALL 16 query blocks of its batch over just those keys:
unnormalized numerators num[g] = exp(S)·V and denominators den[g]. The host
combines the pair during unsharding: out[g] = (num0+num1)/(den0+den1), which
is exact — each core's denominator covers exactly its own key subset.

This removes the K/V projection duplication without any cross-core collective
(a previous AllGather-based variant lost more to the serial CC stream +
launch-skew barrier than the dedup saved; see kernel_cc.py).

Per-core x^T input is packed [own-parity keys | peer-parity keys], so the K/V
projections read contiguous own-key columns while the Q projection consumes
all 2048 (permuted) queries. Packed query block pb attends (pb%8)+1 own key
tiles — a uniform program on every core; the causality difference (whether the
last tile is triangular, open, or fully padded) lives in per-core additive-
mask input data. Softmax skips max-subtraction (scores ~N(0,1)); the
denominator comes free from the Exp activation's accumulate output. P-tile
transposes for the AV matmul run on the DMA xbar (SBUF->SBUF), keeping the PE
on matmuls.
"""

import time

import numpy as np
import ml_dtypes

import concourse.bass as bass
import concourse.bacc as bacc
import concourse.tile as tile
from concourse import mybir
from concourse import bass_utils

BF16 = ml_dtypes.bfloat16
P = 128
B, S, D = 4, 2048, 1024
EC = D // P    # contraction chunks (8)
NGB = 16       # query blocks per core (all global blocks of the batch)
NOT = 8        # own key tiles per core
NCORES = 8
MASKV = -960.0  # additive pre-scale mask; -30 after the 1/sqrt(D) scale

_compiled_nc = None
_runner = None
last_result = None


def _trace_kernel(tc, num_out, den_out, xT, wqT, wkT, wvT, maskadd):
    nc = tc.nc
    f32 = mybir.dt.float32
    bf16 = mybir.dt.bfloat16
    ts = bass.ts

    with (
        tc.tile_pool(name="sb", bufs=1) as sb,
        tc.tile_pool(name="ps", bufs=2, space="PSUM") as ps,
    ):
        # ---- persistent SBUF ----
        xT_s = sb.tile([P, EC, S], bf16)     # packed x^T: [own keys | peer keys]
        KT_s = sb.tile([P, EC, D], bf16)     # K^T of own keys only
        V_s = sb.tile([P, NOT, D], bf16)     # V of own key tiles
        QT_s = sb.tile([P, EC, S], bf16)     # Q^T of all 2048 packed queries
        mask_s = sb.tile([P, NGB, P], f32)

        def load_w(w_dram, nm):
            w_s = sb.tile([P, EC, D], bf16, tag="w", bufs=3, name=nm)
            nc.scalar.dma_start(w_s[:, 0, :P], w_dram[:P, :P])
            nc.scalar.dma_start(w_s[:, 0, P:], w_dram[:P, P:])
            for ec in range(1, EC):
                nc.scalar.dma_start(w_s[:, ec], w_dram[ts(ec, P), :])
            return w_s

        # all loads up front on the ACT hwdge queue; first projection's
        # operands (wk + own-key x columns) land first
        wk_s = sb.tile([P, EC, D], bf16, tag="w", bufs=3, name="wk_s")
        nc.scalar.dma_start(wk_s[:, 0, :P], wkT[:P, :P])
        nc.scalar.dma_start(xT_s[:, 0, :512], xT[:P, :512])
        nc.scalar.dma_start(wk_s[:, 0, P:], wkT[:P, P:])
        nc.scalar.dma_start(xT_s[:, 0, 512:D], xT[:P, 512:D])
        for ec in range(1, EC):
            nc.scalar.dma_start(wk_s[:, ec, :], wkT[ts(ec, P), :])
            nc.scalar.dma_start(xT_s[:, ec, :D], xT[ts(ec, P), :D])
        wv_s = load_w(wvT, "wv_s")
        wq_s = load_w(wqT, "wq_s")
        for ec in range(EC):
            nc.scalar.dma_start(xT_s[:, ec, D:], xT[ts(ec, P), D:])
        for g in range(NGB):
            nc.scalar.dma_start(mask_s[:, g], maskadd[g])

        # ---- K^T projection of OWN keys: KT[d, k] over x cols [0, 1024) ----
        for sc in range(2):
            for dc in range(EC):
                acc = ps.tile([P, 512], f32, tag="s", bufs=3)
                for ec in range(EC):
                    nc.tensor.matmul(
                        acc, wk_s[:, ec, ts(dc, P)], xT_s[:, ec, ts(sc, 512)],
                        start=(ec == 0), stop=(ec == EC - 1))
                nc.scalar.copy(KT_s[:, dc, ts(sc, 512)], acc)

        # ---- V projection of OWN key tiles ----
        for ot in range(NOT):
            acc = ps.tile([P, D], f32, tag="big")
            for ec in range(EC):
                lhsT = xT_s[:, ec, ts(ot, P)]
                for nh in range(2):
                    nc.tensor.matmul(
                        acc[:, ts(nh, 512)], lhsT, wv_s[:, ec, ts(nh, 512)],
                        start=(ec == 0), stop=(ec == EC - 1))
            nc.vector.tensor_copy(V_s[:, ot], acc)

        # ---- Q^T projection of ALL packed queries, 512-col chunks ----
        # chunk order puts the first attention blocks' queries first
        for qc in [1, 3, 0, 2]:
            for dc in range(EC):
                acc = ps.tile([P, 512], f32, tag="s", bufs=3)
                for ec in range(EC):
                    nc.tensor.matmul(
                        acc, wq_s[:, ec, ts(dc, P)], xT_s[:, ec, ts(qc, 512)],
                        start=(ec == 0), stop=(ec == EC - 1))
                nc.scalar.copy(QT_s[:, dc, ts(qc, 512)], acc)

        # ---- partial attention: packed block pb attends (pb%8)+1 own tiles ----
        inv_sqrt_d = 1.0 / float(np.sqrt(D))

        def s_phase(pb):
            n = (pb % 8) + 1
            ncols = n * P
            nch = (ncols + 511) // 512
            p_sb = sb.tile([P, NOT * P], bf16, tag="p_sb", bufs=2)
            pT_sb = sb.tile([P, NOT, P], bf16, tag="pT_sb", bufs=2)
            dsl = sb.tile([P, 2], f32, tag="dsl", bufs=2)
            for ch in range(nch):
                c0 = ch * 512
                cw = min(512, ncols - c0)
                sfull = ps.tile([P, 512], f32, tag="s", bufs=3)
                sps = sfull[:, :cw]
                for dc in range(EC):
                    nc.tensor.matmul(
                        sps, QT_s[:, dc, ts(pb, P)], KT_s[:, dc, c0:c0 + cw],
                        start=(dc == 0), stop=(dc == EC - 1))
                if c0 + cw == ncols:  # last chunk holds the masked tile
                    nc.vector.tensor_add(
                        sps[:, cw - P:cw], sps[:, cw - P:cw], mask_s[:, pb])
                nc.scalar.activation(
                    p_sb[:, c0:c0 + cw], sps,
                    mybir.ActivationFunctionType.Exp,
                    scale=inv_sqrt_d,
                    accum_out=dsl[:, ch:ch + 1])
                # pT_sb[p, kt, q] = p_sb[q, 128*kt + p]
                nc.sync.dma_start(pT_sb[:, ch * 4:ch * 4 + cw // P, :],
                                  p_sb[:, c0:c0 + cw], transpose=True)
            return p_sb, pT_sb, dsl, n, nch

        def av_phase(pb, p_sb, pT_sb, dsl, n, nch):
            den = sb.tile([P, 1], f32, tag="den", bufs=2)
            nc.vector.reduce_sum(den, dsl[:, :nch], axis=mybir.AxisListType.X)
            nc.sync.dma_start(den_out[pb], den)

            acc = ps.tile([P, D], f32, tag="big")
            for kt in range(n):
                for nh in range(2):
                    nc.tensor.matmul(
                        acc[:, ts(nh, 512)], pT_sb[:, kt, :], V_s[:, kt, ts(nh, 512)],
                        start=(kt == 0), stop=(kt == n - 1))
            o_sb = sb.tile([P, D], f32, tag="o_sb", bufs=2)
            # unnormalized numerator out; halves so the DMA overlaps the copy
            nc.vector.tensor_copy(o_sb[:, :512], acc[:, :512])
            nc.sync.dma_start(num_out[pb, :, :512], o_sb[:, :512])
            nc.vector.tensor_copy(o_sb[:, 512:], acc[:, 512:])
            nc.sync.dma_start(num_out[pb, :, 512:], o_sb[:, 512:])

        # big blocks first while pipelining warms; finish on a mid-size block
        # so the last av still has transpose-feed and AV work to cover the tail
        order = [7, 15, 6, 14, 5, 13, 4, 12, 0, 8, 1, 9, 2, 10, 3, 11]
        pending = None
        for pb in order:
            state = s_phase(pb)
            if pending is not None:
                av_phase(*pending)
            pending = (pb,) + state
        av_phase(*pending)


def build_nc(debug=False):
    nc = bacc.Bacc("TRN2", target_bir_lowering=False, debug=debug,
                   enable_asserts=False, num_devices=NCORES)
    bf16 = mybir.dt.bfloat16
    f32 = mybir.dt.float32
    xT = nc.dram_tensor("xT", (D, S), bf16, kind="ExternalInput").ap()
    wqT = nc.dram_tensor("wqT", (D, D), bf16, kind="ExternalInput").ap()
    wkT = nc.dram_tensor("wkT", (D, D), bf16, kind="ExternalInput").ap()
    wvT = nc.dram_tensor("wvT", (D, D), bf16, kind="ExternalInput").ap()
    maskadd = nc.dram_tensor("maskadd", (NGB, P, P), f32,
                             kind="ExternalInput").ap()
    num_out = nc.dram_tensor("num", (NGB, P, D), f32, kind="ExternalOutput").ap()
    den_out = nc.dram_tensor("den", (NGB, P, 1), f32, kind="ExternalOutput").ap()
    with tile.TileContext(nc) as tc:
        _trace_kernel(tc, num_out, den_out, xT, wqT, wkT, wvT, maskadd)
    nc.compile()
    return nc


def _get_compiled():
    global _compiled_nc
    if _compiled_nc is None:
        _compiled_nc = build_nc(debug=False)
    return _compiled_nc


def _get_runner():
    """Jit-once shard_map runner over the 8 NeuronCores."""
    global _runner
    if _runner is not None:
        return _runner
    import jax
    from jax.experimental.shard_map import shard_map
    from jax.sharding import Mesh, PartitionSpec
    from concourse import bass2jax

    nc = _get_compiled()
    bass2jax.install_neuronx_cc_hook()

    partition_name = (nc.partition_id_tensor.name
                      if nc.partition_id_tensor else None)
    in_names, out_names, out_avals, zero_outs = [], [], [], []
    for alloc in nc.m.functions[0].allocations:
        if not isinstance(alloc, mybir.MemoryLocationSet):
            continue
        name = alloc.memorylocations[0].name
        if alloc.kind == "ExternalInput":
            if name != partition_name:
                in_names.append(name)
        elif alloc.kind == "ExternalOutput":
            shape = tuple(alloc.tensor_shape)
            dtype = mybir.dt.np(alloc.dtype)
            out_names.append(name)
            out_avals.append(jax.core.ShapedArray(shape, dtype))
            zero_outs.append(np.zeros(shape, dtype))
    n_params = len(in_names)
    all_in_names = list(in_names) + list(out_names)
    if partition_name is not None:
        all_in_names.append(partition_name)
    donate = tuple(range(n_params, n_params + len(out_names)))

    def _body(*args):
        operands = list(args)
        if partition_name is not None:
            operands.append(bass2jax.partition_id_tensor())
        outs = bass2jax._bass_exec_p.bind(
            *operands,
            out_avals=tuple(out_avals),
            in_names=tuple(all_in_names),
            out_names=tuple(out_names),
            lowering_input_output_aliases=(),
            sim_require_finite=True,
            sim_require_nnan=True,
            nc=nc,
        )
        return tuple(outs)

    devices = jax.devices()[:NCORES]
    mesh = Mesh(np.asarray(devices), ("core",))
    nin = n_params + len(out_names)
    sharded = jax.jit(
        shard_map(_body, mesh=mesh,
                  in_specs=(PartitionSpec("core"),) * nin,
                  out_specs=(PartitionSpec("core"),) * len(out_names),
                  check_rep=False),
        donate_argnums=donate, keep_unused=True)
    _runner = (sharded, in_names, out_names, out_avals, n_params, zero_outs, mesh)
    return _runner


def run_device(in_maps):
    sharded, in_names, out_names, out_avals, n_params, zero_outs, _ = _get_runner()
    concat_in = [
        np.concatenate([np.asarray(in_maps[c][nm]) for c in range(NCORES)], axis=0)
        for nm in in_names
    ]
    concat_zeros = [
        np.zeros((NCORES * z.shape[0], *z.shape[1:]), z.dtype) for z in zero_outs
    ]
    out_arrs = sharded(*concat_in, *concat_zeros)
    return [
        {nm: np.asarray(out_arrs[i]).reshape(NCORES, *out_avals[i].shape)[c]
         for i, nm in enumerate(out_names)}
        for c in range(NCORES)
    ]


def _pb_of(g, h):
    """Packed block index of global query block g on core parity h."""
    if g % 2 == h:
        return (g - h) // 2
    return 8 + (g - (1 - h)) // 2


def make_in_maps(x):
    """Per-core host-side slicing + layout prep (no matmul math here)."""
    x = np.asarray(x, dtype=np.float32)
    r = np.arange(P)
    tri_add = np.where(r[None, :] <= r[:, None], 0.0, MASKV).astype(np.float32)
    # packed block pb's last own tile: pb<8 -> diagonal (triangular);
    # pb>=8 -> h=0: fully open (tile 2q < g), h=1: fully padded (2q+1 > g)
    mask_h = []
    for h in range(2):
        m = np.empty((NGB, P, P), np.float32)
        m[:8] = tri_add
        m[8:] = 0.0 if h == 0 else MASKV
        mask_h.append(m)

    in_maps = []
    for c in range(NCORES):
        b, h = c // 2, c % 2
        blocks = [2 * i + h for i in range(8)] + [2 * i + (1 - h) for i in range(8)]
        xp = np.concatenate([x[b][g * P:(g + 1) * P] for g in blocks], axis=0)
        in_maps.append({
            "xT": np.ascontiguousarray(xp.T).astype(BF16),
            "maskadd": mask_h[h],
        })
    return in_maps


def make_weight_map(inputs):
    """Pre-transposed bf16 weights keyed by NEFF input name."""
    return {
        "wqT": np.ascontiguousarray(np.asarray(inputs["Wq"], np.float32).T).astype(BF16),
        "wkT": np.ascontiguousarray(np.asarray(inputs["Wk"], np.float32).T).astype(BF16),
        "wvT": np.ascontiguousarray(np.asarray(inputs["Wv"], np.float32).T).astype(BF16),
    }


def kernel(x, Wq, bq, Wk, bk, Wv, bv, mask):
    global last_result
    x = np.asarray(x, np.float32)
    Wq = np.asarray(Wq, np.float32)
    Wk = np.asarray(Wk, np.float32)
    Wv = np.asarray(Wv, np.float32)
    bq = np.asarray(bq, np.float32)
    bk = np.asarray(bk, np.float32)
    bv = np.asarray(bv, np.float32)
    mask = np.asarray(mask)

    causal = bool(np.array_equal(mask != 0, np.tril(np.ones(mask.shape, bool))))
    if np.any(bq) or np.any(bk) or not causal:
        return _np_reference(x, Wq, bq, Wk, bk, Wv, bv, mask)

    in_maps = make_in_maps(x)
    wT = make_weight_map({"Wq": Wq, "Wk": Wk, "Wv": Wv})
    for m in in_maps:
        m.update(wT)

    results = None
    for attempt in range(3):  # remote NeuronCores occasionally wedge transiently
        try:
            results = run_device(in_maps)
            break
        except Exception:
            if attempt == 2:
                raise
            time.sleep(30)

    out = np.empty((B * S, D), np.float32)
    for b in range(B):
        r0, r1 = results[2 * b], results[2 * b + 1]
        n0 = np.asarray(r0["num"], np.float32)
        n1 = np.asarray(r1["num"], np.float32)
        d0 = np.asarray(r0["den"], np.float32)
        d1 = np.asarray(r1["den"], np.float32)
        for g in range(NGB):
            p0, p1 = _pb_of(g, 0), _pb_of(g, 1)
            den = d0[p0] + d1[p1]  # [P, 1]
            out[b * S + g * P: b * S + (g + 1) * P] = (n0[p0] + n1[p1]) / den
    if np.any(bv):
        out = out + bv[None, :]  # attn rows sum to 1, so bv adds exactly
    return out


def _np_reference(x, Wq, bq, Wk, bk, Wv, bv, mask):
    outs = []
    for b in range(x.shape[0]):
        xb = x[b]
        Q = xb @ Wq.T + bq
        K = xb @ Wk.T + bk
        V = xb @ Wv.T + bv
        Sc = (Q @ K.T) / np.float32(np.sqrt(x.shape[2]))
        Sc = np.where(mask == 0, np.float32(-1e9), Sc)
        Sc = Sc - Sc.max(axis=1, keepdims=True)
        E = np.exp(Sc)
        A = E / E.sum(axis=1, keepdims=True)
        outs.append(A @ V)
    return np.concatenate(outs, axis=0).astype(np.float32)
